# revision 19
# baseline (speedup 1.0000x reference)
"""Trainium2 Bass kernel for BiAttention (b=8, n=m=1024, d=512).

Sharding: data-parallel over batch — one batch element per NeuronCore,
8 cores, no cross-core communication.

v7 design (all matmul operands bf16; rel-err budget is 2e-2, bf16 lands ~3e-4):

  x1c  = bf16(x1)            [DVE cast]
  x2cw = bf16(x2 * w3bc)     [DVE mul-cast; w3bc = ones-col x w3-row via PE]
  x1T  (d,n) = XBAR(x1c)     [DMA transpose engine — zero PE cost]
  x2Tw (d,m) = XBAR(x2cw) ++ col m=w1
  sim chunk  = x1T_t^T @ x2Tw   -> psum cols [m | s1]
               + logm1 row accumulated onto the s1 column (mask bias)
  E = exp(psum)   (n, vm+1) bf16; col vm = exp(s1+logm1) = g1  (free!)
  ET = XBAR(E[:, :vm])
  x1g = x1 * g1  ++ col d=g1   (g1 folded into U_col rhs; den2 = g-column)
  x2g = x2 * g2  ++ col d=g2   (g2 folded into U_row rhs; den1 = g-column)
  U_col u = E_u^T @ x1g   -> (m, d | den2); Q2C = U_col * (g2/den2)
  U_row t = ET_t^T @ x2g  -> (n, d | den1); c2q = U_row / den1
  V t     = ET_t^T @ Q2C  -> (n, d);  q2c_att = V / den1
  out = [x1, c2q, x1*c2q, x1*q2c_att]           (n, 4d) f32

The PE runs (almost) nothing but the four contractions: every transpose
goes through the DMA XBAR, exp(sim) needs no scaling at all (softmax
weights live in tiny per-tile column scales folded into rhs operands and
evictions), and both softmax denominators fall out of the contraction
psums as the appended g columns.

Input DMAs all ride ONE HWDGE queue (Act) in strict need-order: the
shared DMA engines drain a queue FIFO, so completion order == need order;
half-quad granules put the first usable pair in SBUF ~1us after the
fixed ~8us SPMD preamble.

Mask-suffix specialization: tiles of 128 that are fully masked at the end
of either sequence are skipped in the contractions; the host dispatches to
a NEFF compiled for (kn, km) kept-tile counts.  Partially-masked tiles are
exact via the exponent biases (g=0 rows contribute nothing).
"""

import numpy as np
from contextlib import ExitStack

import concourse.bacc as bacc
import concourse.tile as tile
import concourse.mybir as mybir
from concourse.bass_utils import run_bass_kernel_spmd

F32 = mybir.dt.float32
BF = mybir.dt.bfloat16
U8 = mybir.dt.uint8
EXP = mybir.ActivationFunctionType.Exp
COPY = mybir.ActivationFunctionType.Copy

P = 128
N = 1024          # x1 rows
M = 1024          # x2 rows
D = 512           # feature dim
NT, MT, DC = N // P, M // P, D // P
NEGB = -30000.0   # exp(x + NEGB) == 0.0 exactly for |x| < 80

N_CORES = 8

_CACHE = {}


def _chunks(width, lim=256, last=257):
    """Chunks of `lim`; the final chunk may be up to `last` (PSUM bank max
    512 f32, and the g/s1 column rides in the last chunk)."""
    out = []
    o = 0
    while width - o > last:
        out.append((o, lim))
        o += lim
    out.append((o, width - o))
    return out


def _build(kn, km, dbg=False):
    """Build the kernel keeping the first kn n-tiles / km m-tiles of the
    contractions (tiles beyond that must be fully masked)."""
    vm = km * P  # valid m extent
    nc = bacc.Bacc("TRN2", target_bir_lowering=False, debug=False)
    x1d = nc.dram_tensor("x1", [N, D], F32, kind="ExternalInput").ap()
    x2d = nc.dram_tensor("x2", [M, D], F32, kind="ExternalInput").ap()
    m1d = nc.dram_tensor("x1_mask", [N], U8, kind="ExternalInput").ap()
    m2d = nc.dram_tensor("x2_mask", [M], U8, kind="ExternalInput").ap()
    wd = nc.dram_tensor("W", [3 * D], F32, kind="ExternalInput").ap()
    outd = nc.dram_tensor("out", [N, 4 * D], F32, kind="ExternalOutput").ap()
    if dbg:
        dbg_x2Tw = nc.dram_tensor("dbg_x2Tw", [P, DC * (vm + 1)], F32,
                                  kind="ExternalOutput").ap()
        dbg_E = nc.dram_tensor("dbg_E", [P, NT * (vm + 1)], F32,
                               kind="ExternalOutput").ap()
        dbg_ET = nc.dram_tensor("dbg_ET", [P, km * N], F32,
                                kind="ExternalOutput").ap()
        dbg_g = nc.dram_tensor("dbg_g", [P, km + NT], F32,
                               kind="ExternalOutput").ap()

    x1r_d = x1d.rearrange("(t p) d -> p t d", p=P)
    x2r_d = x2d.rearrange("(t p) d -> p t d", p=P)
    out_r = outd.rearrange("(t p) e -> p t e", p=P)

    # sim psum chunks over the widened (vm+1) extent; last chunk carries s1
    mch = _chunks(vm + 1)

    with tile.TileContext(nc) as tc, ExitStack() as ctx:
        const = ctx.enter_context(tc.tile_pool(name="const", bufs=1))
        big = ctx.enter_context(tc.tile_pool(name="big", bufs=1))
        rows = ctx.enter_context(tc.tile_pool(name="rows", bufs=1))
        work = ctx.enter_context(tc.tile_pool(name="work", bufs=3))
        psA = ctx.enter_context(tc.tile_pool(name="psA", bufs=4, space="PSUM"))
        psB = ctx.enter_context(tc.tile_pool(name="psB", bufs=2, space="PSUM"))
        ps512 = ctx.enter_context(tc.tile_pool(name="ps512", bufs=2, space="PSUM"))

        # ---------- big buffers ----------
        x1n = big.tile([P, NT, D], F32)        # natural x1 (outputs)
        x2n = big.tile([P, km, D], F32)        # natural x2
        x1c = big.tile([P, NT, D], BF)         # bf16 x1 (XBAR source)
        x2cw = big.tile([P, km, D], BF)        # bf16 x2*w3 (XBAR source)
        x1g = big.tile([P, kn, D + 1], BF)     # x1*g1 ++ g1 col
        x2g = big.tile([P, km, D + 1], BF)     # x2*g2 ++ g2 col
        x1T = big.tile([P, DC, N], BF)         # (d_chunk, n)
        # innermost dim padded to a multiple of 128: the XBAR's 3D output
        # requires an aligned middle-dim stride (odd 769 corrupts c>=1)
        x2Tw = big.tile([P, DC, vm + P], BF)   # (d_chunk, m)*w3 ++ w1 col
        E = big.tile([P, NT, vm + 1], BF)      # exp(sim); col vm = g1
        ET = big.tile([P, km, N], BF)          # E^T
        Q2C = big.tile([P, km, D], BF)         # q2c * g2

        # ---------- input DMAs, ALL on the Act HWDGE queue in strict
        # need-order, half-quad granules ----------
        wrow = rows.tile([1, 12 * P], F32)
        nc.scalar.dma_start(wrow[:], wd.rearrange("(a n) -> a n", a=1))
        nc.scalar.dma_start(x1n[:, 0:2, :], x1r_d[:, 0:2, :])
        nc.scalar.dma_start(x1n[:, 2:4, :], x1r_d[:, 2:4, :])
        nc.scalar.dma_start(x2n[:, 0:2, :], x2r_d[:, 0:2, :])
        nc.scalar.dma_start(x2n[:, 2:min(4, km), :], x2r_d[:, 2:min(4, km), :])
        m1row = rows.tile([1, N], U8)
        nc.scalar.dma_start(m1row[:], m1d.rearrange("(a n) -> a n", a=1))
        m2row = rows.tile([1, M], U8)
        nc.scalar.dma_start(m2row[:], m2d.rearrange("(a n) -> a n", a=1))

        # ---------- tiny constants (DVE; ready immediately) ----------
        onesb = const.tile([1, 1], BF)
        nc.vector.memset(onesb[:], 1.0)
        onef = const.tile([1, 1], F32)
        nc.vector.memset(onef[:], 1.0)
        onesrow = const.tile([1, P], BF)
        nc.vector.memset(onesrow[:], 1.0)

        # W row -> columns via PE row->col transposes (warms the PE clock)
        pwc = psA.tile([P, 12], F32, tag="psA")
        for c in range(12):
            nc.tensor.transpose(pwc[:, c:c + 1], wrow[0:1, c * P:(c + 1) * P],
                                onef[0:1, 0:1])
        wcols = const.tile([P, 12], F32)  # (p, c): w1=0:4 w2=4:8 w3=8:12
        nc.vector.tensor_copy(wcols[:], pwc[:])
        w3rec = const.tile([P, 4], F32)
        nc.vector.reciprocal(w3rec[:], wcols[:, 8:12])
        u2r = const.tile([P, 4], BF)      # w2/w3 — recovers s2 from x2Tw
        nc.vector.tensor_mul(u2r[:], wcols[:, 4:8], w3rec[:])

        # w3 broadcast across partitions: w3bc = ones_col x w3_row (PE)
        w3rowb = rows.tile([1, D], BF)
        nc.vector.tensor_copy(w3rowb[:], wrow[0:1, 2 * D:3 * D])
        w3bc = const.tile([P, D], F32)
        pw3 = ps512.tile([P, D], F32, tag="ps512", name="pw3")
        nc.tensor.matmul(pw3[:], onesrow[0:1, :], w3rowb[:],
                         start=True, stop=True)
        nc.vector.tensor_copy(w3bc[:], pw3[:])

        # masks -> exponent-offset rows (0 valid / NEGB padded)
        logm1b = rows.tile([1, N], BF)
        nc.vector.tensor_scalar_mul(logm1b[:], m1row[:], NEGB)
        logm2 = rows.tile([1, vm], F32)
        nc.vector.tensor_scalar_mul(logm2[:], m2row[0:1, 0:vm], NEGB)

        # ---------- cast (DVE) + transpose (DMA XBAR) — zero PE cost.
        # XBARs are issued on the SAME (Act) queue as the input loads, so
        # the queue head never exposes a transpose whose source data is
        # still in flight to the DMA engines (that stalls the pool) ------
        def x1_prep(t):
            nc.vector.tensor_copy(x1c[:, t, :], x1n[:, t, :])
            nc.scalar.dma_start(x1T[:, 0:DC, t * P:(t + 1) * P], x1c[:, t, :],
                                transpose=True)

        def x2_prep(k):
            nc.vector.tensor_mul(x2cw[:, k, :], x2n[:, k, :], w3bc[:])
            nc.scalar.dma_start(x2Tw[:, 0:DC, k * P:(k + 1) * P], x2cw[:, k, :],
                                transpose=True)

        def w1_cols():
            for c in range(DC):
                nc.vector.tensor_copy(x2Tw[:, c, vm:vm + 1],
                                      wcols[:, c:c + 1])

        # ---------- sim -> E (exp evict; s1 col gains mask bias) ----------
        def sim_tile(t, h):
            off, w = mch[h]
            last = off + w == vm + 1
            pool = psB if w > 256 else psA
            pe = pool.tile([P, w], F32, tag=pool.name, name=f"pe_{t}_{h}")
            for c in range(DC):
                nc.tensor.matmul(pe[:],
                                 x1T[:, c, t * P:(t + 1) * P],
                                 x2Tw[:, c, off:off + w],
                                 start=(c == 0), stop=(c == DC - 1 and not last))
            if last:
                # += logm1 on the s1 column only (PE row->col via 1-wide matmul)
                nc.tensor.matmul(pe[:, w - 1:w],
                                 logm1b[0:1, t * P:(t + 1) * P],
                                 onesb[0:1, 0:1],
                                 start=False, stop=True, skip_group_check=True)
            nc.scalar.activation(E[:, t, off:off + w], pe[:], EXP)

        g1c = const.tile([P, NT], F32)

        def x1_gate(t):
            # x1g = x1 * g1 ++ g1 col (fused scale + f32->bf16 cast)
            g1 = g1c[:, t:t + 1]
            nc.vector.tensor_copy(g1, E[:, t, vm:vm + 1])
            nc.vector.tensor_scalar_mul(x1g[:, t, 0:D], x1n[:, t, :], g1)
            nc.vector.tensor_copy(x1g[:, t, D:D + 1], g1)

        # ---------- s2/g2 path, then x2g ----------
        g2c = const.tile([P, km], F32)

        def s2_g2():
            brow = rows.tile([1, vm], F32)
            for h, (off, w) in enumerate(_chunks(vm, lim=512, last=512)):
                ps_s = ps512.tile([1, w], F32, tag="ps512", name=f"ps_b2_{h}")
                for c in range(DC):
                    nc.tensor.matmul(ps_s[:], u2r[:, c:c + 1],
                                     x2Tw[:, c, off:off + w],
                                     start=(c == 0), stop=(c == DC - 1))
                nc.vector.tensor_add(brow[:, off:off + w], ps_s[:],
                                     logm2[:, off:off + w])
            pbc = psA.tile([P, km], F32, tag="psA", name="pbc")
            for k in range(km):
                nc.tensor.transpose(pbc[:, k:k + 1],
                                    brow[0:1, k * P:(k + 1) * P],
                                    onef[0:1, 0:1])
            nc.scalar.activation(g2c[:], pbc[:], EXP)

        def x2_gate(k):
            nc.vector.tensor_scalar_mul(x2g[:, k, 0:D], x2n[:, k, :],
                                        g2c[:, k:k + 1])
            nc.vector.tensor_copy(x2g[:, k, D:D + 1], g2c[:, k:k + 1])

        # ---------- E transposes -> ET via the DMA transpose XBAR ----------
        def e_xpose(t):
            nc.sync.dma_start(ET[:, 0:km, t * P:(t + 1) * P],
                              E[:, t, 0:vm], transpose=True)

        # ---------- schedule: preps as data lands, sim chunk-granular ------
        for t in (0, 1):
            x1_prep(t)
        for t in (2, 3):
            x1_prep(t)
        for k in range(min(2, km)):
            x2_prep(k)
        for k in range(2, min(4, km)):
            x2_prep(k)
        nc.scalar.dma_start(x1n[:, 4:6, :], x1r_d[:, 4:6, :])
        nc.scalar.dma_start(x1n[:, 6:8, :], x1r_d[:, 6:8, :])
        for p in range(2, (km + 1) // 2):
            hi = min(2 * p + 2, km)
            nc.scalar.dma_start(x2n[:, 2 * p:hi, :], x2r_d[:, 2 * p:hi, :])
        w1_cols()
        for t in (0, 1, 2, 3):
            sim_tile(t, 0)
        for t in (4, 5, 6, 7):
            x1_prep(t)
        for k in range(4, km):
            x2_prep(k)
        for t in (0, 1, 2, 3):
            sim_tile(t, 1)
        # out block 0 = x1: one bulk store, now that x1n is fully loaded
        nc.sync.dma_start(out_r[:, :, 0:D], x1n[:])
        for h in range(2, len(mch)):
            for t in (0, 1, 2, 3):
                sim_tile(t, h)
        s2_g2()
        for h in range(len(mch)):
            sim_tile(4, h)
        for t in range(min(4, kn)):
            x1_gate(t)
        for k in range(km):
            x2_gate(k)
        for t in range(0, 4):
            e_xpose(t)
        for h in range(len(mch)):
            sim_tile(5, h)
        e_xpose(4)
        for h in range(len(mch)):
            sim_tile(6, h)
        e_xpose(5)
        for h in range(len(mch)):
            sim_tile(7, h)
        e_xpose(6)
        e_xpose(7)
        for t in range(4, kn):
            x1_gate(t)

        # ---------- U_col -> Q2C (scaled by g2/den2) ----------
        def u_col(u):
            pa = psA.tile([P, 256], F32, tag="psA", name=f"ua_{u}")
            pb = psB.tile([P, 257], F32, tag="psB", name=f"ub_{u}")
            for k in range(kn):
                lhs = E[:, k, u * P:(u + 1) * P]
                nc.tensor.matmul(pa[:], lhs, x1g[:, k, 0:256],
                                 start=(k == 0), stop=(k == kn - 1))
                nc.tensor.matmul(pb[:], lhs, x1g[:, k, 256:513],
                                 start=(k == 0), stop=(k == kn - 1))
            rg = work.tile([P, 1], F32, tag="rg", name=f"rg_{u}")
            nc.vector.reciprocal(rg[:], pb[:, 256:257])
            nc.vector.tensor_mul(rg[:], rg[:], g2c[:, u:u + 1])
            nc.scalar.activation(Q2C[:, u, 0:256], pa[:], COPY, scale=rg[:])
            nc.scalar.activation(Q2C[:, u, 256:512], pb[:, 0:256], COPY,
                                 scale=rg[:])

        if dbg:
            dx = big.tile([P, DC, vm + 1], F32)
            nc.vector.tensor_copy(dx[:], x2Tw[:, :, 0:vm + 1])
            nc.sync.dma_start(dbg_x2Tw.rearrange("p (a e) -> p a e", a=1)[:, 0, :],
                              dx[:].rearrange("p a e -> p (a e)"))
            dE = big.tile([P, NT, vm + 1], F32)
            nc.vector.tensor_copy(dE[:], E[:])
            nc.sync.dma_start(dbg_E.rearrange("p (a e) -> p a e", a=1)[:, 0, :],
                              dE[:].rearrange("p a e -> p (a e)"))
            dT = big.tile([P, km, N], F32)
            nc.vector.tensor_copy(dT[:], ET[:])
            nc.sync.dma_start(dbg_ET.rearrange("p (a e) -> p a e", a=1)[:, 0, :],
                              dT[:].rearrange("p a e -> p (a e)"))
            dg = big.tile([P, km + NT], F32)
            nc.vector.tensor_copy(dg[:, 0:km], g2c[:])
            nc.vector.tensor_copy(dg[:, km:km + NT], g1c[:])
            nc.sync.dma_start(dbg_g.rearrange("p (a e) -> p a e", a=1)[:, 0, :],
                              dg[:])
        for u in range(km):
            u_col(u)

        # ---------- U_row -> c2q ; out blocks 1, 2 ----------
        rden1c = const.tile([P, NT], F32)

        def u_row(t):
            pa = psA.tile([P, 256], F32, tag="psA", name=f"ra_{t}")
            pb = psB.tile([P, 257], F32, tag="psB", name=f"rb_{t}")
            for k in range(km):
                lhs = ET[:, k, t * P:(t + 1) * P]
                nc.tensor.matmul(pa[:], lhs, x2g[:, k, 0:256],
                                 start=(k == 0), stop=(k == km - 1))
                nc.tensor.matmul(pb[:], lhs, x2g[:, k, 256:513],
                                 start=(k == 0), stop=(k == km - 1))
            rd = rden1c[:, t:t + 1]
            nc.vector.reciprocal(rd, pb[:, 256:257])
            combo = work.tile([P, 2 * D], F32, tag="ev", name=f"cb_{t}")
            nc.scalar.activation(combo[:, 0:256], pa[:], COPY, scale=rd)
            nc.scalar.activation(combo[:, 256:512], pb[:, 0:256], COPY,
                                 scale=rd)
            nc.vector.tensor_mul(combo[:, D:2 * D], x1n[:, t, :], combo[:, 0:D])
            nc.sync.dma_start(out_r[:, t, D:3 * D], combo[:])

        # ---------- V -> q2c_att ; out block 3 = x1 . (V*rden1) ----------
        def v_row(t):
            pv = ps512.tile([P, D], F32, tag="ps512", name=f"pv_{t}")
            for k in range(km):
                nc.tensor.matmul(pv[:], ET[:, k, t * P:(t + 1) * P],
                                 Q2C[:, k, :],
                                 start=(k == 0), stop=(k == km - 1))
            vtmp = work.tile([P, D], F32, tag="x1rd", name=f"vt_{t}")
            nc.scalar.activation(vtmp[:], pv[:], COPY, scale=rden1c[:, t:t + 1])
            prod = work.tile([P, D], F32, tag="x1rd", name=f"pv2_{t}")
            eng = nc.gpsimd if t < NT - 2 else nc.vector
            eng.tensor_mul(prod[:], vtmp[:], x1n[:, t, :])
            nc.sync.dma_start(out_r[:, t, 3 * D:4 * D], prod[:])

        # interleave: V(t) only needs rden1c[t] (from u_row(t)) and Q2C
        u_row(0)
        for t in range(1, NT):
            u_row(t)
            v_row(t - 1)
        v_row(NT - 1)

    nc.compile()
    return nc


def _kept_tiles(mask):
    """Tiles (of 128) up to and including the last one with any valid row."""
    valid = ~mask.astype(bool)           # (b, L)
    any_valid = valid.reshape(valid.shape[0], -1, P).any(axis=2).any(axis=0)
    nz = np.nonzero(any_valid)[0]
    return int(nz[-1]) + 1 if len(nz) else 1


def _get_nc(kn, km):
    key = (kn, km)
    if key not in _CACHE:
        _CACHE[key] = _build(kn, km)
    return _CACHE[key]


def _run(inputs, trace=False, trace_cores=None):
    x1 = np.ascontiguousarray(np.asarray(inputs["x1"], dtype=np.float32))
    x2 = np.ascontiguousarray(np.asarray(inputs["x2"], dtype=np.float32))
    m1 = np.ascontiguousarray(np.asarray(inputs["x1_mask"]).astype(np.uint8))
    m2 = np.ascontiguousarray(np.asarray(inputs["x2_mask"]).astype(np.uint8))
    W = np.ascontiguousarray(np.asarray(inputs["W"], dtype=np.float32))
    nc = _get_nc(_kept_tiles(m1), _kept_tiles(m2))
    in_maps = [
        {"x1": x1[i], "x2": x2[i], "x1_mask": m1[i], "x2_mask": m2[i], "W": W}
        for i in range(N_CORES)
    ]
    res = run_bass_kernel_spmd(nc, in_maps, core_ids=list(range(N_CORES)),
                               trace=trace, trace_cores=trace_cores)
    out = np.stack([res.results[i]["out"] for i in range(N_CORES)], axis=0)
    return out.astype(np.float32), res


def kernel(x1, x1_mask, x2, x2_mask, W, bias=None, **_kw):
    # bias is mathematically irrelevant: a global additive constant cancels in
    # both softmaxes, and every output term is softmax-weighted.
    out, _ = _run({"x1": x1, "x1_mask": x1_mask, "x2": x2, "x2_mask": x2_mask,
                   "W": W})
    return out
